# revision 11
# baseline (speedup 1.0000x reference)
"""Causal attentive statistics pooling — Trainium2 Bass kernel.

Strategy (hardcoded for B=8, C=1536, T=4096, A=128, 8 cores):
  - Data-parallel over batch: one sample per NeuronCore.
  - Layout A on-chip: channels on partitions (12 blocks of 128), time on the
    free axis.  All bulk elementwise work in bf16 (DVE 2x mode), prefix sums
    via tensor_tensor_scan (fp32 state), sqrt/square/tanh/exp on ScalarE,
    the three [A,C]@[C,T] matmuls on the PE in bf16.
  - The causal-mean attention term is computed as scan(W1m @ (x*m)) * rcount
    (matmul and prefix-sum commute), so the [C,T] mean tensor is never
    materialized; only causal-std requires the full [C,T] pipeline.
  - Host precomputes tiny per-sample tables: mask row and 1/count row.
  - Phase 2 (tiny): tanh -> logits -> exp -> cumsum -> reciprocal, then
    broadcast e and 1/Z across partitions with K=1 PE matmuls.
  - Phase 3 re-streams x and computes weighted mean/std sums with two more
    scans; final sums come from fused accumulators (tensor_tensor_reduce and
    activation accum_out).
"""

import sys

sys.path.insert(0, "/opt/trn_rl_repo")

from contextlib import ExitStack

import ml_dtypes
import numpy as np

import concourse.bass as bass
import concourse.tile as tile
from concourse import bacc
from concourse import mybir
from concourse.bass_utils import run_bass_kernel_spmd

B, C, T, A = 8, 1536, 4096, 128
P = 128
CB = C // P  # 12 channel blocks
TC = 512  # time chunk
NCH = T // TC  # 8 chunks
EPS = 1e-12
FW = float(1.0 / (T + EPS))

F32 = mybir.dt.float32
BF16 = mybir.dt.bfloat16
ALU = mybir.AluOpType
ACT = mybir.ActivationFunctionType
BF = ml_dtypes.bfloat16

_CACHE = {}


def _bcast_build(nc, pool_ps, out_sb, row_sb, ones_col, j0=0, j1=None):
    """Broadcast a [1, T] row across 128 partitions into out_sb via K=1 PE
    matmuls + ScalarE PSUM->SBUF copies.  Chunks [j0, j1) of the row."""
    if j1 is None:
        j1 = T // TC
    for j in range(j0, j1):
        ps = pool_ps.tile([P, TC], F32, tag="bc")
        nc.tensor.matmul(
            ps[:, :], ones_col[:, :], row_sb[:, j * TC : (j + 1) * TC],
            start=True, stop=True,
        )
        nc.scalar.copy(out_sb[:, (j - j0) * TC : (j - j0 + 1) * TC], ps[:, :])


def build_program():
    nc = bacc.Bacc("TRN2", target_bir_lowering=False, debug=False)

    x_d = nc.dram_tensor("x", [C, T], F32, kind="ExternalInput")
    m_d = nc.dram_tensor("mrow", [1, T], BF16, kind="ExternalInput")
    rc_d = nc.dram_tensor("rcrow", [1, T], BF16, kind="ExternalInput")
    w1x_d = nc.dram_tensor("w1xT", [C, A], BF16, kind="ExternalInput")
    w1m_d = nc.dram_tensor("w1mT", [C, A], BF16, kind="ExternalInput")
    w1s_d = nc.dram_tensor("w1sT", [C, A], BF16, kind="ExternalInput")
    w2_d = nc.dram_tensor("w2col", [A, 1], BF16, kind="ExternalInput")
    b1_d = nc.dram_tensor("b1col", [A, 1], F32, kind="ExternalInput")
    b2_d = nc.dram_tensor("b2val", [1, 1], F32, kind="ExternalInput")
    out_d = nc.dram_tensor("out", [2, CB, P], F32, kind="ExternalOutput")

    x_r = x_d.rearrange("(k p) t -> p k t", p=P)
    out_r = out_d.rearrange("s k p -> s p k")

    with tile.TileContext(nc) as tc, ExitStack() as ctx:
        const = ctx.enter_context(tc.tile_pool(name="const", bufs=1))
        xpool = ctx.enter_context(tc.tile_pool(name="xpool", bufs=2))
        dbl = ctx.enter_context(tc.tile_pool(name="dbl", bufs=1))
        stdp = ctx.enter_context(tc.tile_pool(name="stdp", bufs=2))
        psum = ctx.enter_context(tc.tile_pool(name="psum", bufs=2, space="PSUM"))
        psbc = ctx.enter_context(tc.tile_pool(name="psbc", bufs=2, space="PSUM"))

        def bcslc(tbl, t0):
            return (
                tbl[:, t0 : t0 + TC]
                .rearrange("p (o t) -> p o t", o=1)
                .broadcast_to([P, CB, TC])
            )

        # ---- constants / weights ----
        w1x_sb = const.tile([P, CB, A], BF16)
        w1m_sb = const.tile([P, CB, A], BF16)
        w1s_sb = const.tile([P, CB, A], BF16)
        nc.sync.dma_start(w1x_sb[:], w1x_d.rearrange("(k p) m -> p k m", p=P))
        nc.sync.dma_start(w1m_sb[:], w1m_d.rearrange("(k p) m -> p k m", p=P))
        nc.sync.dma_start(w1s_sb[:], w1s_d.rearrange("(k p) m -> p k m", p=P))
        w2_sb = const.tile([A, 1], BF16)
        b1_sb = const.tile([A, 1], F32)
        b2_sb = const.tile([1, 1], F32)
        nc.sync.dma_start(w2_sb[:], w2_d.ap())
        nc.sync.dma_start(b1_sb[:], b1_d.ap())
        nc.sync.dma_start(b2_sb[:], b2_d.ap())
        mrow_sb = const.tile([1, T], BF16)
        rcrow_sb = const.tile([1, T], BF16)
        nc.sync.dma_start(mrow_sb[:], m_d.ap())
        nc.sync.dma_start(rcrow_sb[:], rc_d.ap())
        ones_col = const.tile([1, P], BF16)
        nc.vector.memset(ones_col[:], 1.0)

        mB = const.tile([P, T // 2], BF16, tag="mB")  # only t >= T/2 can be masked
        rcB = const.tile([P, T], BF16, tag="rcB")
        _bcast_build(nc, psbc, mB, mrow_sb, ones_col, j0=NCH // 2)
        _bcast_build(nc, psbc, rcB, rcrow_sb, ones_col)

        # carries and accumulators
        cxcar = const.tile([P, CB, 1], F32)
        cxxcar = const.tile([P, CB, 1], F32)
        s1car = const.tile([P, CB, 1], F32)
        vcar = const.tile([P, CB, 1], F32)
        ymcar = const.tile([P, 1], F32)
        fm_acc = const.tile([P, CB], F32)
        fs_acc = const.tile([P, CB], F32)
        fs_stage = const.tile([P, CB], F32)
        fm_stage = const.tile([P, CB], F32)
        nc.vector.memset(fs_acc[:], 0.0)

        z_sb = const.tile([P, T], BF16, tag="z")

        # ================= PHASE 1 =================
        for ch in range(NCH):
            t0 = ch * TC
            masked = ch >= NCH // 2

            xbf = xpool.tile([P, CB, TC], BF16, tag="xbf")
            # tiny same-engine write absorbs the WAR waits so the SWDGE DMA
            # itself needs few sync-wait commands (HW limit)
            nc.gpsimd.memset(xbf[:, :, 0:1], 0.0)
            nc.gpsimd.dma_start(xbf[:], x_r[:, :, t0 : t0 + TC])

            rcslc = bcslc(rcB, t0)

            if masked:
                mslc = bcslc(mB, t0 - T // 2)
                xm = dbl.tile([P, CB, TC], BF16, tag="xm")
                nc.vector.tensor_mul(xm[:], xbf[:], mslc)
            else:
                xm = xbf

            sq = dbl.tile([P, CB, TC], BF16, tag="sq")
            nc.vector.tensor_mul(sq[:], xm[:], xbf[:])

            cx = dbl.tile([P, CB, TC], BF16, tag="cx")
            cxx = dbl.tile([P, CB, TC], BF16, tag="cxx")
            for k in range(CB):
                init = 0.0 if ch == 0 else cxcar[:, k, :]
                nc.vector.tensor_tensor_scan(
                    cx[:, k, :], xm[:, k, :], xm[:, k, :], init, ALU.add, ALU.bypass
                )
            nc.vector.tensor_copy(cxcar[:], cx[:, :, TC - 1 : TC])
            for k in range(CB):
                init = 0.0 if ch == 0 else cxxcar[:, k, :]
                nc.vector.tensor_tensor_scan(
                    cxx[:, k, :], sq[:, k, :], sq[:, k, :], init, ALU.add, ALU.bypass
                )
            nc.vector.tensor_copy(cxxcar[:], cxx[:, :, TC - 1 : TC])

            # mean in-place into cx; b = cxx*rc in-place into cxx
            nc.vector.tensor_mul(cx[:], cx[:], rcslc)
            nc.vector.tensor_mul(cxx[:], cxx[:], rcslc)
            mm = dbl.tile([P, CB, TC], BF16, tag="sq")  # reuse sq slot
            nc.scalar.activation(mm[:], cx[:], ACT.Square)
            nc.vector.tensor_sub(cxx[:], cxx[:], mm[:])  # v = b - mean^2
            nc.vector.tensor_scalar(cxx[:], cxx[:], EPS, None, ALU.max)  # clamp
            std = stdp.tile([P, CB, TC], BF16, tag="std")
            nc.scalar.activation(std[:], cxx[:], ACT.Sqrt)

            # PE: z_chunk = W1x @ x + W1s @ std  (accumulate), ym = W1m @ xm
            zc = psum.tile([P, TC], F32, tag="zc")
            for k in range(CB):
                nc.tensor.matmul(
                    zc[:, :], w1x_sb[:, k, :], xbf[:, k, :],
                    start=(k == 0), stop=False,
                )
            for k in range(CB):
                nc.tensor.matmul(
                    zc[:, :], w1s_sb[:, k, :], std[:, k, :],
                    start=False, stop=(k == CB - 1),
                )
            ym = psum.tile([P, TC], F32, tag="ym")
            for k in range(CB):
                nc.tensor.matmul(
                    ym[:, :], w1m_sb[:, k, :], xm[:, k, :],
                    start=(k == 0), stop=(k == CB - 1),
                )

            # scan the mean-term along T (small: [A, TC]) then scale by rcount
            zms = dbl.tile([P, TC], BF16, tag="zms")
            init = 0.0 if ch == 0 else ymcar[:, :]
            # data1 must not be PSUM; it is ignored under op1=bypass
            nc.vector.tensor_tensor_scan(
                zms[:, :], ym[:, :], rcB[:, t0 : t0 + TC], init, ALU.add, ALU.bypass
            )
            nc.vector.tensor_copy(ymcar[:], zms[:, TC - 1 : TC])
            nc.vector.tensor_mul(zms[:, :], zms[:, :], rcB[:, t0 : t0 + TC])
            # z = zc + zms
            nc.vector.tensor_add(z_sb[:, t0 : t0 + TC], zc[:, :], zms[:, :])

        # ================= PHASE 2 =================
        nc.scalar.activation(z_sb[:], z_sb[:], ACT.Tanh, bias=b1_sb[:, 0:1], scale=1.0)

        ebf_row = const.tile([1, T], BF16, tag="ebfrow")
        for j in range(T // TC):
            lg = psum.tile([1, TC], F32, tag="zc")
            nc.tensor.matmul(
                lg[:, :], w2_sb[:, :], z_sb[:, j * TC : (j + 1) * TC],
                start=True, stop=True,
            )
            nc.scalar.activation(
                ebf_row[:, j * TC : (j + 1) * TC], lg[:, :], ACT.Exp,
                bias=b2_sb[:, 0:1], scale=1.0,
            )
        z_row = const.tile([1, T], F32, tag="zrow")
        nc.vector.tensor_tensor_scan(
            z_row[:, :], ebf_row[:, :], ebf_row[:, :], 0.0, ALU.add, ALU.bypass
        )
        rzbf_row = const.tile([1, T], BF16, tag="rzbfrow")
        with nc.allow_low_precision(reason="1/Z used in bf16 products anyway"):
            nc.vector.reciprocal(rzbf_row[:], z_row[:])

        eB = const.tile([P, T], BF16, tag="rcB")  # reuse rcB slot
        rzB = const.tile([P, T], BF16, tag="rcB2")
        _bcast_build(nc, psbc, eB, ebf_row, ones_col)
        _bcast_build(nc, psbc, rzB, rzbf_row, ones_col)

        # ================= PHASE 3 =================
        for ch in range(NCH):
            t0 = ch * TC

            xbf = xpool.tile([P, CB, TC], BF16, tag="xbf")
            # tiny same-engine write absorbs the WAR waits so the SWDGE DMA
            # itself needs few sync-wait commands (HW limit)
            nc.gpsimd.memset(xbf[:, :, 0:1], 0.0)
            nc.gpsimd.dma_start(xbf[:], x_r[:, :, t0 : t0 + TC])

            eslc = bcslc(eB, t0)
            rzslc = bcslc(rzB, t0)

            g = dbl.tile([P, CB, TC], BF16, tag="sq")
            nc.vector.tensor_mul(g[:], xbf[:], eslc)
            s1 = dbl.tile([P, CB, TC], BF16, tag="cx")
            for k in range(CB):
                init = 0.0 if ch == 0 else s1car[:, k, :]
                nc.vector.tensor_tensor_scan(
                    s1[:, k, :], g[:, k, :], g[:, k, :], init, ALU.add, ALU.bypass
                )
            nc.vector.tensor_copy(s1car[:], s1[:, :, TC - 1 : TC])

            # wm = s1 * rz (in-place) with fused per-block row sums
            # (tensor_tensor_reduce is broken on this runtime; use stt+accum)
            for k in range(CB):
                nc.vector.scalar_tensor_tensor(
                    out=s1[:, k, :], in0=s1[:, k, :], scalar=1.0,
                    in1=rzB[:, t0 : t0 + TC], op0=ALU.mult, op1=ALU.mult,
                    accum_out=fm_stage[:, k : k + 1],
                )
            if ch == 0:
                nc.vector.tensor_copy(fm_acc[:], fm_stage[:])
            else:
                nc.vector.tensor_add(fm_acc[:], fm_acc[:], fm_stage[:])

            d = dbl.tile([P, CB, TC], BF16, tag="xm")
            nc.vector.tensor_sub(d[:], xbf[:], s1[:])
            dd = dbl.tile([P, CB, TC], BF16, tag="cxx")
            nc.scalar.activation(dd[:], d[:], ACT.Square)
            nc.vector.tensor_mul(dd[:], dd[:], eslc)  # edd in-place
            v_t = stdp.tile([P, CB, TC], BF16, tag="std")
            for k in range(CB):
                init = 0.0 if ch == 0 else vcar[:, k, :]
                nc.vector.tensor_tensor_scan(
                    v_t[:, k, :], dd[:, k, :], dd[:, k, :], init, ALU.add, ALU.bypass
                )
            nc.vector.tensor_copy(vcar[:], v_t[:, :, TC - 1 : TC])
            nc.vector.tensor_mul(v_t[:], v_t[:], rzslc)  # wvar in-place

            wstd = dbl.tile([P, CB, TC], BF16, tag="xm")  # reuse d slot
            for k in range(CB):
                nc.scalar.activation(
                    wstd[:, k, :], v_t[:, k, :], ACT.Sqrt,
                    accum_out=fs_stage[:, k : k + 1],
                )
            nc.vector.tensor_add(fs_acc[:], fs_acc[:], fs_stage[:])

        # ================= FINALIZE =================
        nc.vector.tensor_scalar(fm_acc[:], fm_acc[:], FW, None, ALU.mult)
        nc.vector.tensor_scalar(fs_acc[:], fs_acc[:], FW, None, ALU.mult)
        nc.sync.dma_start(out_r[0], fm_acc[:])
        nc.sync.dma_start(out_r[1], fs_acc[:])

    nc.finalize()  # run Bacc passes (wait splitting, reg alloc) before serialize
    return nc


def _get_program():
    if "nc" not in _CACHE:
        _CACHE["nc"] = build_program()
    return _CACHE["nc"]


def kernel(x, lengths, W1, b1, W2, b2):
    x = np.asarray(x, dtype=np.float32)
    lengths = np.asarray(lengths)
    W1 = np.asarray(W1, dtype=np.float32)
    b1 = np.asarray(b1, dtype=np.float32)
    W2 = np.asarray(W2, dtype=np.float32)
    b2 = np.asarray(b2, dtype=np.float32)

    nc = _get_program()

    # host-side tiny tables
    t_idx = np.arange(T)
    w1xT = np.ascontiguousarray(W1[:, 0:C].T).astype(BF)
    w1mT = np.ascontiguousarray(W1[:, C : 2 * C].T).astype(BF)
    w1sT = np.ascontiguousarray(W1[:, 2 * C : 3 * C].T).astype(BF)
    w2col = np.ascontiguousarray(W2.T).astype(BF)  # [A, 1]
    b1col = b1.reshape(A, 1).astype(np.float32)
    b2val = b2.reshape(1, 1).astype(np.float32)

    in_maps = []
    for b in range(B):
        ln = int(lengths[b])
        m = (t_idx < ln).astype(np.float32)
        count = np.clip(np.cumsum(m), 1.0, None)
        rc = (1.0 / count).astype(BF).reshape(1, T)
        in_maps.append(
            {
                "x": np.ascontiguousarray(x[b]),
                "mrow": m.astype(BF).reshape(1, T),
                "rcrow": rc,
                "w1xT": w1xT,
                "w1mT": w1mT,
                "w1sT": w1sT,
                "w2col": w2col,
                "b1col": b1col,
                "b2val": b2val,
            }
        )

    import os

    trace = bool(os.environ.get("BASS_KERNEL_TRACE"))
    res = run_bass_kernel_spmd(nc, in_maps, core_ids=list(range(B)), trace=trace)
    _CACHE["exec_time_ns"] = getattr(res, "exec_time_ns", None)
    _CACHE["results_obj"] = res
    outs = []
    for b in range(B):
        o = np.asarray(res.results[b]["out"], dtype=np.float32)  # [2, CB, P]
        mean = o[0].reshape(C)  # c = k*128 + p
        std = o[1].reshape(C)
        outs.append(np.concatenate([mean, std]))
    return np.stack(outs).astype(np.float32)


# revision 13
# speedup vs baseline: 1.0909x; 1.0909x over previous
"""Causal attentive statistics pooling — Trainium2 Bass kernel (v2).

Strategy (hardcoded for B=8, C=1536, T=4096, A=128, 8 cores):
  - Data-parallel over batch: one sample per NeuronCore.
  - Layout: channels on partitions (12 blocks of 128), time on the free axis.
    Bulk elementwise in bf16 (DVE 2x), prefix ops via tensor_tensor_scan.
  - Key trick: the running mean / running normalized sums are computed with a
    single ratio-recurrence scan  state_t = (d0_t + state_{t-1}) * rho_t
    where rho = count_{t-1}/count_t (resp. Z_{t-1}/Z_t) is an fp32 broadcast
    table.  This emits mean, E[x^2], weighted-mean, and weighted-var directly
    from the scan with no separate [C,T]-sized multiply passes.
  - The causal-mean attention term uses scan(W1m @ (x*m/count_prev)) (matmul
    and column-scaled prefix-sum commute), so mean is never an input to PE.
  - Squares run on ScalarE; sqrt with fused row-sum accumulators produces the
    final std sums; weighted-mean sums come from ScalarE copy+accumulate.
"""

import sys

sys.path.insert(0, "/opt/trn_rl_repo")

from contextlib import ExitStack

import ml_dtypes
import numpy as np

import concourse.bass as bass
import concourse.tile as tile
from concourse import bacc
from concourse import mybir
from concourse.bass_utils import run_bass_kernel_spmd

B, C, T, A = 8, 1536, 4096, 128
P = 128
CB = C // P  # channel blocks
TC = 512  # time chunk
NCH = T // TC
EPS = 1e-12
FW = float(1.0 / (T + EPS))

F32 = mybir.dt.float32
BF16 = mybir.dt.bfloat16
ALU = mybir.AluOpType
ACT = mybir.ActivationFunctionType
BF = ml_dtypes.bfloat16

_CACHE = {}


def build_program():
    FOLD = T // P
    nc = bacc.Bacc("TRN2", target_bir_lowering=False, debug=False)
    scr_d = nc.dram_tensor("zscratch", [1, T], F32)

    x_d = nc.dram_tensor("x", [C, T], F32, kind="ExternalInput")
    mrc_d = nc.dram_tensor("mrcrow", [1, T], BF16, kind="ExternalInput")
    cp_d = nc.dram_tensor("cprow", [1, T], BF16, kind="ExternalInput")
    rhoc_d = nc.dram_tensor("rhocrow", [1, T], F32, kind="ExternalInput")
    w1x_d = nc.dram_tensor("w1xT", [C, A], BF16, kind="ExternalInput")
    w1m_d = nc.dram_tensor("w1mT", [C, A], BF16, kind="ExternalInput")
    w1s_d = nc.dram_tensor("w1sT", [C, A], BF16, kind="ExternalInput")
    w2_d = nc.dram_tensor("w2col", [A, 1], BF16, kind="ExternalInput")
    b1_d = nc.dram_tensor("b1col", [A, 1], F32, kind="ExternalInput")
    b2_d = nc.dram_tensor("b2val", [1, 1], F32, kind="ExternalInput")
    out_d = nc.dram_tensor("out", [2, CB, P], F32, kind="ExternalOutput")

    x_r = x_d.rearrange("(k p) t -> p k t", p=P)
    out_r = out_d.rearrange("s k p -> s p k")

    with tile.TileContext(nc) as tc, ExitStack() as ctx:
        const = ctx.enter_context(tc.tile_pool(name="const", bufs=1))
        xpool = ctx.enter_context(tc.tile_pool(name="xpool", bufs=2))
        dbl = ctx.enter_context(tc.tile_pool(name="dbl", bufs=1))
        stdp = ctx.enter_context(tc.tile_pool(name="stdp", bufs=2))
        psum = ctx.enter_context(tc.tile_pool(name="psum", bufs=2, space="PSUM"))
        psbc = ctx.enter_context(tc.tile_pool(name="psbc", bufs=2, space="PSUM"))

        def bcslc(tbl, t0):
            return (
                tbl[:, t0 : t0 + TC]
                .rearrange("p (o t) -> p o t", o=1)
                .broadcast_to([P, CB, TC])
            )

        def bcast_build(out_sb, row_sb, copy_dtype_note=None):
            """[1, T] row -> [128, T] via K=1 PE matmuls + ScalarE copies."""
            for j in range(T // TC):
                ps = psbc.tile([P, TC], F32, tag="bc")
                nc.tensor.matmul(
                    ps[:, :], ones_col[:, :], row_sb[:, j * TC : (j + 1) * TC],
                    start=True, stop=True,
                )
                nc.scalar.copy(out_sb[:, j * TC : (j + 1) * TC], ps[:, :])

        # ---- weights / host tables ----
        w1x_sb = const.tile([P, CB, A], BF16)
        w1m_sb = const.tile([P, CB, A], BF16)
        w1s_sb = const.tile([P, CB, A], BF16)
        nc.sync.dma_start(w1x_sb[:], w1x_d.rearrange("(k p) m -> p k m", p=P))
        nc.sync.dma_start(w1m_sb[:], w1m_d.rearrange("(k p) m -> p k m", p=P))
        nc.sync.dma_start(w1s_sb[:], w1s_d.rearrange("(k p) m -> p k m", p=P))
        w2_sb = const.tile([A, 1], BF16)
        b1_sb = const.tile([A, 1], F32)
        b2_sb = const.tile([1, 1], F32)
        nc.sync.dma_start(w2_sb[:], w2_d.ap())
        nc.sync.dma_start(b1_sb[:], b1_d.ap())
        nc.sync.dma_start(b2_sb[:], b2_d.ap())
        ones_col = const.tile([1, P], BF16)
        nc.vector.memset(ones_col[:], 1.0)
        ones_colf = const.tile([1, P], F32)
        nc.vector.memset(ones_colf[:], 1.0)

        # broadcast tables: mrcB/cpB bf16; rhoB f32 (shared phase1/phase3)
        mrcB = const.tile([P, T], BF16, tag="tblA")  # m/count_prev, later e~
        cpB = const.tile([P, T], BF16, tag="tblB")  # count_prev
        rhoB = const.tile([P, T], F32, tag="tblR")  # rho_c, later rho_z

        mrc_row = dbl.tile([1, T], BF16, tag="sqx")
        cp_row = dbl.tile([1, T], BF16, tag="xt")
        rhoc_row = dbl.tile([1, T], F32, tag="mean")
        nc.sync.dma_start(mrc_row[:], mrc_d.ap())
        nc.sync.dma_start(cp_row[:], cp_d.ap())
        nc.sync.dma_start(rhoc_row[:], rhoc_d.ap())
        bcast_build(mrcB, mrc_row)
        bcast_build(cpB, cp_row)
        for j in range(T // TC):
            ps = psbc.tile([P, TC], F32, tag="bc")
            nc.tensor.matmul(
                ps[:, :], ones_colf[:, :], rhoc_row[:, j * TC : (j + 1) * TC],
                start=True, stop=True,
            )
            nc.scalar.copy(rhoB[:, j * TC : (j + 1) * TC], ps[:, :])

        # carries and accumulators
        meancar = const.tile([P, CB, 1], F32)
        bcar = const.tile([P, CB, 1], F32)
        wmcar = const.tile([P, CB, 1], F32)
        wvcar = const.tile([P, CB, 1], F32)
        ymcar = const.tile([P, 1], F32)
        fm_acc = const.tile([P, CB], F32)
        fs_acc = const.tile([P, CB], F32)
        fm_stage = const.tile([P, CB], F32)
        fs_stage = const.tile([P, CB], F32)
        nc.vector.memset(fs_acc[:], 0.0)
        nc.vector.memset(fm_acc[:], 0.0)

        z_sb = const.tile([P, T], BF16, tag="z")

        # ================= PHASE 1 =================
        for ch in range(NCH):
            t0 = ch * TC

            xbf = xpool.tile([P, CB, TC], BF16, tag="xbf")
            # tiny same-engine write absorbs WAR waits (DMA sync-wait limit)
            nc.gpsimd.memset(xbf[:, :, 0:1], 0.0)
            nc.gpsimd.dma_start(xbf[:], x_r[:, :, t0 : t0 + TC])

            # xt = x * m / count_prev  (mask folded into the table)
            xt = dbl.tile([P, CB, TC], BF16, tag="xt")
            nc.vector.tensor_mul(xt[:], xbf[:], bcslc(mrcB, t0))
            # xxt = xt^2 * count_prev = x^2 m / count_prev
            sqx = dbl.tile([P, CB, TC], BF16, tag="sqx")
            nc.scalar.activation(sqx[:], xt[:], ACT.Square)
            nc.vector.tensor_mul(sqx[:], sqx[:], bcslc(cpB, t0))

            rho2d = rhoB[:, t0 : t0 + TC]
            mean = dbl.tile([P, CB, TC], BF16, tag="mean")
            bm2 = dbl.tile([P, CB, TC], BF16, tag="b")
            for k in range(CB):
                init = 0.0 if ch == 0 else meancar[:, k, :]
                nc.vector.tensor_tensor_scan(
                    mean[:, k, :], xt[:, k, :], rho2d, init, ALU.add, ALU.mult
                )
            nc.vector.tensor_copy(meancar[:], mean[:, :, TC - 1 : TC])
            for k in range(CB):
                init = 0.0 if ch == 0 else bcar[:, k, :]
                nc.vector.tensor_tensor_scan(
                    bm2[:, k, :], sqx[:, k, :], rho2d, init, ALU.add, ALU.mult
                )
            nc.vector.tensor_copy(bcar[:], bm2[:, :, TC - 1 : TC])

            # var = clamp(b - mean^2), std = sqrt
            mm = dbl.tile([P, CB, TC], BF16, tag="sqx")  # reuse
            nc.scalar.activation(mm[:], mean[:], ACT.Square)
            nc.vector.tensor_sub(bm2[:], bm2[:], mm[:])
            nc.vector.tensor_scalar(bm2[:], bm2[:], EPS, None, ALU.max)
            std = stdp.tile([P, CB, TC], BF16, tag="std")
            nc.scalar.activation(std[:], bm2[:], ACT.Sqrt)

            # PE: zc = W1x @ x + W1s @ std ; ym = W1m @ xt
            zc = psum.tile([P, TC], F32, tag="zc")
            for k in range(CB):
                nc.tensor.matmul(
                    zc[:, :], w1x_sb[:, k, :], xbf[:, k, :],
                    start=(k == 0), stop=False,
                )
            for k in range(CB):
                nc.tensor.matmul(
                    zc[:, :], w1s_sb[:, k, :], std[:, k, :],
                    start=False, stop=(k == CB - 1),
                )
            ym = psum.tile([P, TC], F32, tag="ym")
            for k in range(CB):
                nc.tensor.matmul(
                    ym[:, :], w1m_sb[:, k, :], xt[:, k, :],
                    start=(k == 0), stop=(k == CB - 1),
                )

            # mean-feature: scan(ym; rho_c) directly (column scaling commutes)
            zms = const.tile([P, TC], BF16, tag="zms")
            init = 0.0 if ch == 0 else ymcar[:, :]
            nc.vector.tensor_tensor_scan(
                zms[:, :], ym[:, :], rho2d, init, ALU.add, ALU.mult
            )
            nc.vector.tensor_copy(ymcar[:], zms[:, TC - 1 : TC])
            nc.vector.tensor_add(z_sb[:, t0 : t0 + TC], zc[:, :], zms[:, :])

        # ================= PHASE 2 =================
        nc.scalar.activation(z_sb[:], z_sb[:], ACT.Tanh, bias=b1_sb[:, 0:1], scale=1.0)

        ebf_row = dbl.tile([1, T], BF16, tag="sqx")
        for j in range(T // TC):
            lg = psum.tile([1, TC], F32, tag="zc")
            nc.tensor.matmul(
                lg[:, :], w2_sb[:, :], z_sb[:, j * TC : (j + 1) * TC],
                start=True, stop=True,
            )
            nc.scalar.activation(
                ebf_row[:, j * TC : (j + 1) * TC], lg[:, :], ACT.Exp,
                bias=b2_sb[:, 0:1], scale=1.0,
            )
        z_row = dbl.tile([1, T], F32, tag="mean")
        nc.vector.tensor_tensor_scan(
            z_row[:, :], ebf_row[:, :], ebf_row[:, :], 0.0, ALU.add, ALU.bypass
        )
        # folded reciprocal: [1, T] -> DRAM -> [128, FOLD] -> recip -> back
        scr_r = scr_d.rearrange("o (p f) -> (o p) f", p=P)
        nc.sync.dma_start(scr_d.ap(), z_row[:, :])
        zfold = const.tile([P, FOLD], F32, tag="zfold")
        nc.sync.dma_start(zfold[:], scr_r)
        rzfold = const.tile([P, FOLD], F32, tag="rzfold")
        nc.vector.reciprocal(rzfold[:], zfold[:])
        nc.sync.dma_start(scr_r, rzfold[:])
        rz_row = dbl.tile([1, T], F32, tag="b")
        nc.sync.dma_start(rz_row[:, :], scr_d.ap())

        # rho_z row: rho[0]=rz[0], rho[t]=Z[t-1]*rz[t]
        rhoz_row = dbl.tile([1, T], F32, tag="xt")
        nc.vector.tensor_mul(rhoz_row[:, 1:T], z_row[:, 0 : T - 1], rz_row[:, 1:T])
        nc.vector.tensor_copy(rhoz_row[:, 0:1], rz_row[:, 0:1])
        # e~ row: et[0]=e[0], et[t]=e[t]*rz[t-1]
        et_row = dbl.tile([1, T], BF16, tag="zms2")
        nc.vector.tensor_mul(et_row[:, 1:T], ebf_row[:, 1:T], rz_row[:, 0 : T - 1])
        nc.vector.tensor_copy(et_row[:, 0:1], ebf_row[:, 0:1])

        etB = const.tile([P, T], BF16, tag="tblA")  # reuse mrcB slot
        bcast_build(etB, et_row)
        for j in range(T // TC):
            ps = psbc.tile([P, TC], F32, tag="bc")
            nc.tensor.matmul(
                ps[:, :], ones_colf[:, :], rhoz_row[:, j * TC : (j + 1) * TC],
                start=True, stop=True,
            )
            nc.scalar.copy(rhoB[:, j * TC : (j + 1) * TC], ps[:, :])

        # ================= PHASE 3 =================
        for ch in range(NCH):
            t0 = ch * TC

            xbf = xpool.tile([P, CB, TC], BF16, tag="xbf")
            nc.gpsimd.memset(xbf[:, :, 0:1], 0.0)
            nc.gpsimd.dma_start(xbf[:], x_r[:, :, t0 : t0 + TC])

            rho2d = rhoB[:, t0 : t0 + TC]

            gt = dbl.tile([P, CB, TC], BF16, tag="xt")
            nc.vector.tensor_mul(gt[:], xbf[:], bcslc(etB, t0))
            wm = dbl.tile([P, CB, TC], BF16, tag="mean")
            for k in range(CB):
                init = 0.0 if ch == 0 else wmcar[:, k, :]
                nc.vector.tensor_tensor_scan(
                    wm[:, k, :], gt[:, k, :], rho2d, init, ALU.add, ALU.mult
                )
            nc.vector.tensor_copy(wmcar[:], wm[:, :, TC - 1 : TC])

            # fm partial sums via ScalarE copy+accumulate
            for k in range(CB):
                nc.scalar.activation(
                    gt[:, k, :], wm[:, k, :], ACT.Copy,
                    accum_out=fm_stage[:, k : k + 1],
                )
            nc.vector.tensor_add(fm_acc[:], fm_acc[:], fm_stage[:])

            d = dbl.tile([P, CB, TC], BF16, tag="sqx")
            nc.vector.tensor_sub(d[:], xbf[:], wm[:])
            dd = dbl.tile([P, CB, TC], BF16, tag="b")
            nc.scalar.activation(dd[:], d[:], ACT.Square)
            nc.vector.tensor_mul(dd[:], dd[:], bcslc(etB, t0))  # e~ * d^2
            wvar = stdp.tile([P, CB, TC], BF16, tag="std")
            for k in range(CB):
                init = 0.0 if ch == 0 else wvcar[:, k, :]
                nc.vector.tensor_tensor_scan(
                    wvar[:, k, :], dd[:, k, :], rho2d, init, ALU.add, ALU.mult
                )
            nc.vector.tensor_copy(wvcar[:], wvar[:, :, TC - 1 : TC])

            wstd = dbl.tile([P, CB, TC], BF16, tag="mean")  # dummy out
            for k in range(CB):
                nc.scalar.activation(
                    wstd[:, k, :], wvar[:, k, :], ACT.Sqrt,
                    accum_out=fs_stage[:, k : k + 1],
                )
            nc.vector.tensor_add(fs_acc[:], fs_acc[:], fs_stage[:])

        # ================= FINALIZE =================
        nc.vector.tensor_scalar(fm_acc[:], fm_acc[:], FW, None, ALU.mult)
        nc.vector.tensor_scalar(fs_acc[:], fs_acc[:], FW, None, ALU.mult)
        nc.sync.dma_start(out_r[0], fm_acc[:])
        nc.sync.dma_start(out_r[1], fs_acc[:])

    nc.finalize()
    return nc


def _get_program():
    if "nc" not in _CACHE:
        _CACHE["nc"] = build_program()
    return _CACHE["nc"]


def host_tables(ln, Tdim):
    """Per-sample tables: m/count_prev (bf16), count_prev (bf16),
    count_prev/count (f32)."""
    t = np.arange(Tdim)
    m = (t < ln).astype(np.float64)
    count = np.clip(np.cumsum(m), 1.0, None)
    cprev = np.concatenate([[1.0], count[:-1]])
    mrc = (m / cprev).astype(BF).reshape(1, Tdim)
    cp = cprev.astype(BF).reshape(1, Tdim)
    rhoc = (cprev / count).astype(np.float32).reshape(1, Tdim)
    return mrc, cp, rhoc


def make_in_map(xb, ln, W1, b1, W2, b2, Cdim, Tdim):
    mrc, cp, rhoc = host_tables(ln, Tdim)
    return {
        "x": np.ascontiguousarray(xb),
        "mrcrow": mrc,
        "cprow": cp,
        "rhocrow": rhoc,
        "w1xT": np.ascontiguousarray(W1[:, 0:Cdim].T).astype(BF),
        "w1mT": np.ascontiguousarray(W1[:, Cdim : 2 * Cdim].T).astype(BF),
        "w1sT": np.ascontiguousarray(W1[:, 2 * Cdim : 3 * Cdim].T).astype(BF),
        "w2col": np.ascontiguousarray(W2.T).astype(BF),
        "b1col": b1.reshape(A, 1).astype(np.float32),
        "b2val": b2.reshape(1, 1).astype(np.float32),
    }


def kernel(x, lengths, W1, b1, W2, b2):
    x = np.asarray(x, dtype=np.float32)
    lengths = np.asarray(lengths)
    W1 = np.asarray(W1, dtype=np.float32)
    b1 = np.asarray(b1, dtype=np.float32)
    W2 = np.asarray(W2, dtype=np.float32)
    b2 = np.asarray(b2, dtype=np.float32)

    nc = _get_program()
    in_maps = [
        make_in_map(x[b], int(lengths[b]), W1, b1, W2, b2, C, T) for b in range(B)
    ]

    import os

    trace = bool(os.environ.get("BASS_KERNEL_TRACE"))
    res = run_bass_kernel_spmd(nc, in_maps, core_ids=list(range(B)), trace=trace)
    _CACHE["exec_time_ns"] = getattr(res, "exec_time_ns", None)
    _CACHE["results_obj"] = res

    outs = []
    for b in range(B):
        o = np.asarray(res.results[b]["out"], dtype=np.float32)
        outs.append(np.concatenate([o[0].reshape(C), o[1].reshape(C)]))
    return np.stack(outs).astype(np.float32)


# revision 14
# speedup vs baseline: 1.1161x; 1.0231x over previous
"""Causal attentive statistics pooling — Trainium2 Bass kernel (v2).

Strategy (hardcoded for B=8, C=1536, T=4096, A=128, 8 cores):
  - Data-parallel over batch: one sample per NeuronCore.
  - Layout: channels on partitions (12 blocks of 128), time on the free axis.
    Bulk elementwise in bf16 (DVE 2x), prefix ops via tensor_tensor_scan.
  - Key trick: the running mean / running normalized sums are computed with a
    single ratio-recurrence scan  state_t = (d0_t + state_{t-1}) * rho_t
    where rho = count_{t-1}/count_t (resp. Z_{t-1}/Z_t) is an fp32 broadcast
    table.  This emits mean, E[x^2], weighted-mean, and weighted-var directly
    from the scan with no separate [C,T]-sized multiply passes.
  - The causal-mean attention term uses scan(W1m @ (x*m/count_prev)) (matmul
    and column-scaled prefix-sum commute), so mean is never an input to PE.
  - Squares run on ScalarE; sqrt with fused row-sum accumulators produces the
    final std sums; weighted-mean sums come from ScalarE copy+accumulate.
"""

import sys

sys.path.insert(0, "/opt/trn_rl_repo")

from contextlib import ExitStack

import ml_dtypes
import numpy as np

import concourse.bass as bass
import concourse.tile as tile
from concourse import bacc
from concourse import mybir
from concourse.bass_utils import run_bass_kernel_spmd

B, C, T, A = 8, 1536, 4096, 128
P = 128
CB = C // P  # channel blocks
TC = 512  # time chunk
NCH = T // TC
EPS = 1e-12
FW = float(1.0 / (T + EPS))

F32 = mybir.dt.float32
BF16 = mybir.dt.bfloat16
ALU = mybir.AluOpType
ACT = mybir.ActivationFunctionType
BF = ml_dtypes.bfloat16

_CACHE = {}


def build_program():
    FOLD = T // P
    nc = bacc.Bacc("TRN2", target_bir_lowering=False, debug=False)
    scr_d = nc.dram_tensor("zscratch", [1, T], F32)

    x_d = nc.dram_tensor("x", [C, T], F32, kind="ExternalInput")
    mrc_d = nc.dram_tensor("mrcrow", [1, T], BF16, kind="ExternalInput")
    cp_d = nc.dram_tensor("cprow", [1, T], BF16, kind="ExternalInput")
    rhoc_d = nc.dram_tensor("rhocrow", [1, T], F32, kind="ExternalInput")
    w1x_d = nc.dram_tensor("w1xT", [C, A], BF16, kind="ExternalInput")
    w1m_d = nc.dram_tensor("w1mT", [C, A], BF16, kind="ExternalInput")
    w1s_d = nc.dram_tensor("w1sT", [C, A], BF16, kind="ExternalInput")
    w2_d = nc.dram_tensor("w2col", [A, 1], BF16, kind="ExternalInput")
    b1_d = nc.dram_tensor("b1col", [A, 1], F32, kind="ExternalInput")
    b2_d = nc.dram_tensor("b2val", [1, 1], F32, kind="ExternalInput")
    out_d = nc.dram_tensor("out", [2, CB, P], F32, kind="ExternalOutput")

    x_r = x_d.rearrange("(k p) t -> p k t", p=P)
    out_r = out_d.rearrange("s k p -> s p k")

    with tile.TileContext(nc) as tc, ExitStack() as ctx:
        const = ctx.enter_context(tc.tile_pool(name="const", bufs=1))
        xpool = ctx.enter_context(tc.tile_pool(name="xpool", bufs=2))
        dbl = ctx.enter_context(tc.tile_pool(name="dbl", bufs=1))
        stdp = ctx.enter_context(tc.tile_pool(name="stdp", bufs=2))
        hot = ctx.enter_context(tc.tile_pool(name="hot", bufs=2))
        psum = ctx.enter_context(tc.tile_pool(name="psum", bufs=2, space="PSUM"))
        psbc = ctx.enter_context(tc.tile_pool(name="psbc", bufs=2, space="PSUM"))

        def bcslc(tbl, t0):
            return (
                tbl[:, t0 : t0 + TC]
                .rearrange("p (o t) -> p o t", o=1)
                .broadcast_to([P, CB, TC])
            )

        def bcast_build(out_sb, row_sb):
            """[1, T] row -> [128, T] via K=1 PE matmuls + ACT/DVE copies."""
            for j in range(T // TC):
                ps = psbc.tile([P, TC], F32, tag="bc")
                nc.tensor.matmul(
                    ps[:, :], ones_col[:, :], row_sb[:, j * TC : (j + 1) * TC],
                    start=True, stop=True,
                )
                dst = out_sb[:, j * TC : (j + 1) * TC]
                if j % 2 == 0:
                    nc.scalar.copy(dst, ps[:, :])
                else:
                    nc.vector.tensor_copy(dst, ps[:, :])

        # ---- weights / host tables ----
        w1x_sb = const.tile([P, CB, A], BF16)
        w1m_sb = const.tile([P, CB, A], BF16)
        w1s_sb = const.tile([P, CB, A], BF16)
        nc.sync.dma_start(w1x_sb[:], w1x_d.rearrange("(k p) m -> p k m", p=P))
        nc.sync.dma_start(w1m_sb[:], w1m_d.rearrange("(k p) m -> p k m", p=P))
        nc.sync.dma_start(w1s_sb[:], w1s_d.rearrange("(k p) m -> p k m", p=P))
        w2_sb = const.tile([A, 1], BF16)
        b1_sb = const.tile([A, 1], F32)
        b2_sb = const.tile([1, 1], F32)
        nc.sync.dma_start(w2_sb[:], w2_d.ap())
        nc.sync.dma_start(b1_sb[:], b1_d.ap())
        nc.sync.dma_start(b2_sb[:], b2_d.ap())
        ones_col = const.tile([1, P], BF16)
        nc.vector.memset(ones_col[:], 1.0)
        ones_colf = const.tile([1, P], F32)
        nc.vector.memset(ones_colf[:], 1.0)

        # broadcast tables: mrcB/cpB bf16; rhoB f32 (shared phase1/phase3)
        mrcB = const.tile([P, T], BF16, tag="tblA")  # m/count_prev, later e~
        cpB = const.tile([P, T], BF16, tag="tblB")  # count_prev
        rhoB = const.tile([P, T], F32, tag="tblR")  # rho_c, later rho_z

        mrc_row = dbl.tile([1, T], BF16, tag="sqx")
        cp_row = dbl.tile([1, T], BF16, tag="xt")
        rhoc_row = hot.tile([1, T], F32, tag="mean")
        nc.sync.dma_start(mrc_row[:], mrc_d.ap())
        nc.sync.dma_start(cp_row[:], cp_d.ap())
        nc.sync.dma_start(rhoc_row[:], rhoc_d.ap())
        bcast_build(mrcB, mrc_row)
        bcast_build(cpB, cp_row)
        for j in range(T // TC):
            ps = psbc.tile([P, TC], F32, tag="bc")
            nc.tensor.matmul(
                ps[:, :], ones_colf[:, :], rhoc_row[:, j * TC : (j + 1) * TC],
                start=True, stop=True,
            )
            if j % 2 == 0:
                nc.scalar.copy(rhoB[:, j * TC : (j + 1) * TC], ps[:, :])
            else:
                nc.vector.tensor_copy(rhoB[:, j * TC : (j + 1) * TC], ps[:, :])

        # carries and accumulators
        meancar = const.tile([P, CB, 1], F32)
        bcar = const.tile([P, CB, 1], F32)
        wmcar = const.tile([P, CB, 1], F32)
        wvcar = const.tile([P, CB, 1], F32)
        ymcar = const.tile([P, 1], F32)
        fm_acc = const.tile([P, CB], F32)
        fs_acc = const.tile([P, CB], F32)
        fm_stage = const.tile([P, CB], F32)
        fs_stage = const.tile([P, CB], F32)
        nc.vector.memset(fs_acc[:], 0.0)
        nc.vector.memset(fm_acc[:], 0.0)

        z_sb = const.tile([P, T], BF16, tag="z")

        # ================= PHASE 1 =================
        for ch in range(NCH):
            t0 = ch * TC

            xbf = xpool.tile([P, CB, TC], BF16, tag="xbf")
            # tiny same-engine write absorbs WAR waits (DMA sync-wait limit)
            nc.gpsimd.memset(xbf[:, :, 0:1], 0.0)
            nc.gpsimd.dma_start(xbf[:], x_r[:, :, t0 : t0 + TC])

            # xt = x * m / count_prev  (mask folded into the table)
            xt = dbl.tile([P, CB, TC], BF16, tag="xt")
            nc.vector.tensor_mul(xt[:], xbf[:], bcslc(mrcB, t0))
            # xxt = xt^2 * count_prev = x^2 m / count_prev
            sqx = dbl.tile([P, CB, TC], BF16, tag="sqx")
            nc.scalar.activation(sqx[:], xt[:], ACT.Square)
            nc.vector.tensor_mul(sqx[:], sqx[:], bcslc(cpB, t0))

            rho2d = rhoB[:, t0 : t0 + TC]
            mean = hot.tile([P, CB, TC], BF16, tag="mean")
            bm2 = hot.tile([P, CB, TC], BF16, tag="b")
            for k in range(CB):
                init = 0.0 if ch == 0 else meancar[:, k, :]
                nc.vector.tensor_tensor_scan(
                    mean[:, k, :], xt[:, k, :], rho2d, init, ALU.add, ALU.mult
                )
            nc.vector.tensor_copy(meancar[:], mean[:, :, TC - 1 : TC])
            for k in range(CB):
                init = 0.0 if ch == 0 else bcar[:, k, :]
                nc.vector.tensor_tensor_scan(
                    bm2[:, k, :], sqx[:, k, :], rho2d, init, ALU.add, ALU.mult
                )
            nc.vector.tensor_copy(bcar[:], bm2[:, :, TC - 1 : TC])

            # var = clamp(b - mean^2), std = sqrt
            mm = dbl.tile([P, CB, TC], BF16, tag="sqx")  # reuse
            nc.scalar.activation(mm[:], mean[:], ACT.Square)
            nc.vector.tensor_sub(bm2[:], bm2[:], mm[:])
            nc.vector.tensor_scalar(bm2[:], bm2[:], EPS, None, ALU.max)
            std = stdp.tile([P, CB, TC], BF16, tag="std")
            nc.scalar.activation(std[:], bm2[:], ACT.Sqrt)

            # PE: zc = W1x @ x + W1s @ std ; ym = W1m @ xt
            zc = psum.tile([P, TC], F32, tag="zc")
            for k in range(CB):
                nc.tensor.matmul(
                    zc[:, :], w1x_sb[:, k, :], xbf[:, k, :],
                    start=(k == 0), stop=False,
                )
            for k in range(CB):
                nc.tensor.matmul(
                    zc[:, :], w1s_sb[:, k, :], std[:, k, :],
                    start=False, stop=(k == CB - 1),
                )
            ym = psum.tile([P, TC], F32, tag="ym")
            for k in range(CB):
                nc.tensor.matmul(
                    ym[:, :], w1m_sb[:, k, :], xt[:, k, :],
                    start=(k == 0), stop=(k == CB - 1),
                )

            # mean-feature: scan(ym; rho_c) directly (column scaling commutes)
            zms = const.tile([P, TC], BF16, tag="zms")
            init = 0.0 if ch == 0 else ymcar[:, :]
            nc.vector.tensor_tensor_scan(
                zms[:, :], ym[:, :], rho2d, init, ALU.add, ALU.mult
            )
            nc.vector.tensor_copy(ymcar[:], zms[:, TC - 1 : TC])
            nc.vector.tensor_add(z_sb[:, t0 : t0 + TC], zc[:, :], zms[:, :])

        # ================= PHASE 2 =================
        nc.scalar.activation(z_sb[:], z_sb[:], ACT.Tanh, bias=b1_sb[:, 0:1], scale=1.0)

        ebf_row = dbl.tile([1, T], BF16, tag="sqx")
        for j in range(T // TC):
            lg = psum.tile([1, TC], F32, tag="zc")
            nc.tensor.matmul(
                lg[:, :], w2_sb[:, :], z_sb[:, j * TC : (j + 1) * TC],
                start=True, stop=True,
            )
            nc.scalar.activation(
                ebf_row[:, j * TC : (j + 1) * TC], lg[:, :], ACT.Exp,
                bias=b2_sb[:, 0:1], scale=1.0,
            )
        z_row = hot.tile([1, T], F32, tag="mean")
        nc.vector.tensor_tensor_scan(
            z_row[:, :], ebf_row[:, :], ebf_row[:, :], 0.0, ALU.add, ALU.bypass
        )
        # folded reciprocal: [1, T] -> DRAM -> [128, FOLD] -> recip -> back
        scr_r = scr_d.rearrange("o (p f) -> (o p) f", p=P)
        nc.sync.dma_start(scr_d.ap(), z_row[:, :])
        zfold = const.tile([P, FOLD], F32, tag="zfold")
        nc.sync.dma_start(zfold[:], scr_r)
        rzfold = const.tile([P, FOLD], F32, tag="rzfold")
        nc.vector.reciprocal(rzfold[:], zfold[:])
        nc.sync.dma_start(scr_r, rzfold[:])
        rz_row = hot.tile([1, T], F32, tag="b")
        nc.sync.dma_start(rz_row[:, :], scr_d.ap())

        # rho_z row: rho[0]=rz[0], rho[t]=Z[t-1]*rz[t]
        rhoz_row = dbl.tile([1, T], F32, tag="xt")
        nc.vector.tensor_mul(rhoz_row[:, 1:T], z_row[:, 0 : T - 1], rz_row[:, 1:T])
        nc.vector.tensor_copy(rhoz_row[:, 0:1], rz_row[:, 0:1])
        # e~ row: et[0]=e[0], et[t]=e[t]*rz[t-1]
        et_row = dbl.tile([1, T], BF16, tag="zms2")
        nc.vector.tensor_mul(et_row[:, 1:T], ebf_row[:, 1:T], rz_row[:, 0 : T - 1])
        nc.vector.tensor_copy(et_row[:, 0:1], ebf_row[:, 0:1])

        etB = const.tile([P, T], BF16, tag="tblA")  # reuse mrcB slot
        bcast_build(etB, et_row)
        for j in range(T // TC):
            ps = psbc.tile([P, TC], F32, tag="bc")
            nc.tensor.matmul(
                ps[:, :], ones_colf[:, :], rhoz_row[:, j * TC : (j + 1) * TC],
                start=True, stop=True,
            )
            if j % 2 == 0:
                nc.scalar.copy(rhoB[:, j * TC : (j + 1) * TC], ps[:, :])
            else:
                nc.vector.tensor_copy(rhoB[:, j * TC : (j + 1) * TC], ps[:, :])

        # ================= PHASE 3 =================
        for ch in range(NCH):
            t0 = ch * TC

            xbf = xpool.tile([P, CB, TC], BF16, tag="xbf")
            nc.gpsimd.memset(xbf[:, :, 0:1], 0.0)
            nc.gpsimd.dma_start(xbf[:], x_r[:, :, t0 : t0 + TC])

            rho2d = rhoB[:, t0 : t0 + TC]

            gt = dbl.tile([P, CB, TC], BF16, tag="xt")
            nc.vector.tensor_mul(gt[:], xbf[:], bcslc(etB, t0))
            wm = hot.tile([P, CB, TC], BF16, tag="mean")
            for k in range(CB):
                init = 0.0 if ch == 0 else wmcar[:, k, :]
                nc.vector.tensor_tensor_scan(
                    wm[:, k, :], gt[:, k, :], rho2d, init, ALU.add, ALU.mult
                )
            nc.vector.tensor_copy(wmcar[:], wm[:, :, TC - 1 : TC])

            # fm partial sums via ScalarE copy+accumulate
            for k in range(CB):
                nc.scalar.activation(
                    gt[:, k, :], wm[:, k, :], ACT.Copy,
                    accum_out=fm_stage[:, k : k + 1],
                )
            nc.vector.tensor_add(fm_acc[:], fm_acc[:], fm_stage[:])

            d = dbl.tile([P, CB, TC], BF16, tag="sqx")
            nc.vector.tensor_sub(d[:], xbf[:], wm[:])
            dd = hot.tile([P, CB, TC], BF16, tag="b")
            nc.scalar.activation(dd[:], d[:], ACT.Square)
            nc.vector.tensor_mul(dd[:], dd[:], bcslc(etB, t0))  # e~ * d^2
            wvar = stdp.tile([P, CB, TC], BF16, tag="std")
            for k in range(CB):
                init = 0.0 if ch == 0 else wvcar[:, k, :]
                nc.vector.tensor_tensor_scan(
                    wvar[:, k, :], dd[:, k, :], rho2d, init, ALU.add, ALU.mult
                )
            nc.vector.tensor_copy(wvcar[:], wvar[:, :, TC - 1 : TC])

            wstd = hot.tile([P, CB, TC], BF16, tag="mean")  # dummy out
            for k in range(CB):
                nc.scalar.activation(
                    wstd[:, k, :], wvar[:, k, :], ACT.Sqrt,
                    accum_out=fs_stage[:, k : k + 1],
                )
            nc.vector.tensor_add(fs_acc[:], fs_acc[:], fs_stage[:])

        # ================= FINALIZE =================
        nc.vector.tensor_scalar(fm_acc[:], fm_acc[:], FW, None, ALU.mult)
        nc.vector.tensor_scalar(fs_acc[:], fs_acc[:], FW, None, ALU.mult)
        nc.sync.dma_start(out_r[0], fm_acc[:])
        nc.sync.dma_start(out_r[1], fs_acc[:])

    nc.finalize()
    return nc


def _get_program():
    if "nc" not in _CACHE:
        _CACHE["nc"] = build_program()
    return _CACHE["nc"]


def host_tables(ln, Tdim):
    """Per-sample tables: m/count_prev (bf16), count_prev (bf16),
    count_prev/count (f32)."""
    t = np.arange(Tdim)
    m = (t < ln).astype(np.float64)
    count = np.clip(np.cumsum(m), 1.0, None)
    cprev = np.concatenate([[1.0], count[:-1]])
    mrc = (m / cprev).astype(BF).reshape(1, Tdim)
    cp = cprev.astype(BF).reshape(1, Tdim)
    rhoc = (cprev / count).astype(np.float32).reshape(1, Tdim)
    return mrc, cp, rhoc


def make_in_map(xb, ln, W1, b1, W2, b2, Cdim, Tdim):
    mrc, cp, rhoc = host_tables(ln, Tdim)
    return {
        "x": np.ascontiguousarray(xb),
        "mrcrow": mrc,
        "cprow": cp,
        "rhocrow": rhoc,
        "w1xT": np.ascontiguousarray(W1[:, 0:Cdim].T).astype(BF),
        "w1mT": np.ascontiguousarray(W1[:, Cdim : 2 * Cdim].T).astype(BF),
        "w1sT": np.ascontiguousarray(W1[:, 2 * Cdim : 3 * Cdim].T).astype(BF),
        "w2col": np.ascontiguousarray(W2.T).astype(BF),
        "b1col": b1.reshape(A, 1).astype(np.float32),
        "b2val": b2.reshape(1, 1).astype(np.float32),
    }


def kernel(x, lengths, W1, b1, W2, b2):
    x = np.asarray(x, dtype=np.float32)
    lengths = np.asarray(lengths)
    W1 = np.asarray(W1, dtype=np.float32)
    b1 = np.asarray(b1, dtype=np.float32)
    W2 = np.asarray(W2, dtype=np.float32)
    b2 = np.asarray(b2, dtype=np.float32)

    nc = _get_program()
    in_maps = [
        make_in_map(x[b], int(lengths[b]), W1, b1, W2, b2, C, T) for b in range(B)
    ]

    import os

    trace = bool(os.environ.get("BASS_KERNEL_TRACE"))
    res = run_bass_kernel_spmd(nc, in_maps, core_ids=list(range(B)), trace=trace)
    _CACHE["exec_time_ns"] = getattr(res, "exec_time_ns", None)
    _CACHE["results_obj"] = res

    outs = []
    for b in range(B):
        o = np.asarray(res.results[b]["out"], dtype=np.float32)
        outs.append(np.concatenate([o[0].reshape(C), o[1].reshape(C)]))
    return np.stack(outs).astype(np.float32)
